# revision 29
# baseline (speedup 1.0000x reference)
"""GAT (3-layer, heads=1) on a fixed circulant graph, sharded across 8 trn2 cores.

Graph: node i aggregates from sources (i + 131*k) % 8192, k = 0..16 (incl. self).
Since gcd(131, 8192) = 1, relabeling nodes a -> node 131*a turns the graph into a
banded matrix with source offsets 0..16: neighbors become consecutive rows.  The
host permutes inputs into that space, shards 1024 consecutive rows per core with
a 48-row halo (16 per GAT layer, computed redundantly), and un-permutes at the
end.  No collectives needed.

Per-core device pipeline, all in the permuted space:
  emb:    h0T = (x W_emb + b)^T                      (PE, fp32r)
  gat l:  stage A: [h | h a_src | h a_dst] = h_in W_cat per 128-node tile
          (lhsT = h_inT tile, rhs = W_cat) -> node-major H (bf16) + s,d columns
          window: s column -> flat DRAM (node order), read back [128, 9, 17]
          windows w2win[p,t,k] = s[128t+p+k]; attention softmax in a handful
          of wide DVE ops -> coefs C (bf16)
          coefs -> DRAM banded matrix via skewed DMA; read back node-major,
          PE-transpose to at0[src,dst] (+ batched at1 halo transposes)
          aggregate (bf16): layers 1-2 swapped (lhsT=H tile, rhs=at0) so the
          psum lands feat-major -> activation adds bias, writes hT directly;
          layer 3 classic (lhsT=at0, rhs=H) -> node-major rows for the store
"""
import contextlib
import os
import sys

import numpy as np

sys.path.insert(0, "/opt/trn_rl_repo")

N = 8192
OFFSTEP = 131
K17 = 17          # neighbors incl. self
IN, HID, OUT = 512, 256, 128
NCORES = 8
SH = N // NCORES  # 1024 output rows per core
HALO = 16
RA_EMB = SH + 3 * HALO   # 1072 rows of x / h0
RA = [RA_EMB, SH + 2 * HALO, SH + HALO]   # stage-A rows for gat1..3
NT = 9                   # 128-row tiles per core (last partial)
RSK = 160                # banded-matrix row stride (elems; 144 used + 16 zero pad)
SFLAT = 1184             # flat s buffer (1168 used + pad)

_CACHE = {}


def _build(repeat=1, skip=()):
    skip = set(skip) | set((os.environ.get("BENCH_SKIP") or "").split(","))
    import concourse.bass as bass
    import concourse.tile as tile
    from concourse import bacc, mybir
    from concourse.masks import make_identity

    F32 = mybir.dt.float32
    F32R = mybir.dt.float32r
    BF16 = mybir.dt.bfloat16
    AT = mybir.AluOpType
    ACTF = mybir.ActivationFunctionType

    nc = bacc.Bacc("TRN2", target_bir_lowering=False, debug=False)

    xT = nc.dram_tensor("xT", [128, 4, RA_EMB], F32R, kind="ExternalInput")
    wemb = nc.dram_tensor("wemb", [128, 4, HID], F32R, kind="ExternalInput")
    bemb = nc.dram_tensor("bemb", [1, HID], F32, kind="ExternalInput")
    wc1 = nc.dram_tensor("wc1", [128, 2, HID + 2], F32R, kind="ExternalInput")
    wc2 = nc.dram_tensor("wc2", [128, 2, HID + 2], F32R, kind="ExternalInput")
    wc3 = nc.dram_tensor("wc3", [128, 2, HID + 2], F32R, kind="ExternalInput")
    b1 = nc.dram_tensor("b1", [1, HID], F32, kind="ExternalInput")
    b2 = nc.dram_tensor("b2", [1, HID], F32, kind="ExternalInput")
    b3 = nc.dram_tensor("b3", [1, OUT], F32, kind="ExternalInput")
    out_d = nc.dram_tensor("out", [SH, OUT], F32, kind="ExternalOutput")

    sd_flat = nc.dram_tensor("sd_flat", [2, SFLAT], F32, kind="Internal")
    cs = nc.dram_tensor("cs", [NT * 128, RSK], BF16, kind="Internal")

    with tile.TileContext(nc) as tc:
        with (
            tc.tile_pool(name="sing", bufs=1) as sing,
            tc.tile_pool(name="at", bufs=4) as atp,
            tc.tile_pool(name="ps", bufs=2, space="PSUM") as psp,
        ):
          # ---- persistent tiles
          xT_sb = sing.tile([128, 4, RA_EMB], F32R)
          h0T = sing.tile([128, 2, RA_EMB], F32R)
          h1T = sing.tile([128, 2, NT * 128], F32R)
          h2T = sing.tile([128, 2, NT * 128], F32R)
          H_nm = sing.tile([128, NT + 1, HID], BF16)
          sd2 = sing.tile([128, NT, 1], F32)
          w2 = sing.tile([128, 1057], F32)
          E = sing.tile([128, NT, K17], F32)
          Cc = sing.tile([128, NT, K17], BF16)
          ssum = sing.tile([128, NT, 1], F32)
          rs = sing.tile([128, NT, 1], F32)
          a_nm = sing.tile([128, NT, 128], BF16)
          sdT_row = sing.tile([2, SFLAT], F32)
          a_nmh = sing.tile([128, 3, 96], BF16)
          at1a = sing.tile([16, NT, 128], BF16)
          wemb_sb = sing.tile([128, 4, HID], F32R)
          wc_sb = [sing.tile([128, 2, HID + 2], F32R, name=f"wc{l}")
                   for l in range(3)]
          bemb_f = sing.tile([128, 2], F32)
          bf = [sing.tile([128, 2], F32, name=f"bf{l}") for l in range(2)]
          b3_bc = sing.tile([128, OUT], F32)
          ident = sing.tile([128, 128], F32)
          identb = sing.tile([128, 128], BF16)
          out_sb = sing.tile([128, SH // 128, OUT], F32)
          zero_bf = sing.tile([128, NT * RSK], BF16)
          zero_f = sing.tile([1, 256], F32)

          # ---- compute-engine init (runs while first DMAs stream)
          make_identity(nc, ident[:])
          nc.vector.tensor_copy(identb[:], ident[:])
          nc.gpsimd.memset(zero_bf[:], 0.0)
          nc.gpsimd.memset(zero_f[:], 0.0)
          nc.gpsimd.memset(H_nm[:], 0.0)


          loop = tc.For_i(0, repeat, 1) if repeat > 1 else contextlib.nullcontext()
          with loop:
            # ---- load x (feat-major) + first-needed weights ahead of the rest
            nc.sync.dma_start(
                out=xT_sb[:, :, 0:512],
                in_=bass.AP(tensor=xT, offset=0,
                            ap=[[4 * RA_EMB, 128], [RA_EMB, 4], [1, 512]]))
            nc.sync.dma_start(out=wemb_sb[:], in_=wemb.ap())
            nc.sync.dma_start(
                out=bemb_f[:],
                in_=bass.AP(tensor=bemb, offset=0, ap=[[1, 128], [128, 2]]))
            nc.sync.dma_start(
                out=xT_sb[:, :, 512:RA_EMB],
                in_=bass.AP(tensor=xT, offset=512,
                            ap=[[4 * RA_EMB, 128], [RA_EMB, 4],
                                [1, RA_EMB - 512]]))
            for l, wd in enumerate([wc1, wc2, wc3]):
                nc.sync.dma_start(out=wc_sb[l][:], in_=wd.ap())
            # off-band zeros for the banded matrix + s_flat tail
            nc.sync.dma_start(
                out=bass.AP(tensor=cs, offset=0,
                            ap=[[RSK, NT * 128], [1, RSK]]),
                in_=zero_bf[:])
            nc.sync.dma_start(
                out=bass.AP(tensor=sd_flat, offset=RA_EMB,
                            ap=[[SFLAT, 2], [1, SFLAT - RA_EMB]]),
                in_=zero_f[0:1, 0:2 * (SFLAT - RA_EMB)])
            for l, bd in enumerate([b1, b2]):
                nc.sync.dma_start(
                    out=bf[l][:],
                    in_=bass.AP(tensor=bd, offset=0, ap=[[1, 128], [128, 2]]))
            nc.sync.dma_start(
                out=b3_bc[:],
                in_=bass.AP(tensor=b3, offset=0, ap=[[0, 128], [1, OUT]]))

            def emit_psd(l, i):
                # s,d rows for layer l, slice i: psum [2, nw] -> SBUF ->
                # sd_flat rows (s at row 0, d at row 1), fat packets
                ra_l = RA[l]
                n0 = 512 * i
                if n0 >= ra_l:
                    return
                nw = min(512, ra_l - n0)
                Fh_l = OUT if l == 2 else HID
                hT_l = [h0T, h1T, h2T][l]
                pst = psp.tile([1, 512], F32, name="ps_S", tag="ps_S", bufs=1)
                for kc in range(2):
                    nc.tensor.matmul(
                        pst[0:1, 0:nw],
                        wc_sb[l][:, kc, Fh_l:Fh_l + 1],
                        hT_l[:, kc, n0:n0 + nw],
                        start=(kc == 0), stop=(kc == 1))
                with tc.high_priority():
                    nc.scalar.activation(sdT_row[0:1, n0:n0 + nw],
                                         pst[0:1, 0:nw], ACTF.Identity)
                    nc.sync.dma_start(
                        out=bass.AP(tensor=sd_flat, offset=n0,
                                    ap=[[1, nw]]),
                        in_=sdT_row[0:1, n0:n0 + nw])

            def emit_windows():
                # w2[p, c] = s[p + c]; d2win[p, t] = d[128 t + p]
                with tc.high_priority():
                    nc.sync.dma_start(
                        out=w2[:],
                        in_=bass.AP(tensor=sd_flat, offset=0,
                                    ap=[[1, 128], [1, 1057]]))

            # ---- emb: h0T[f, n] = sum_fi W[fi, f] x[fi, n]  (+ bias)
            nsl = [(0, 512), (512, 512), (1024, RA_EMB - 1024)]
            for i, (n0, nw) in enumerate(nsl) if "emb" not in skip else []:
                for m in range(2):
                    pe = psp.tile([128, 512], F32, name="ps_A", tag="ps_A")
                    for kc in range(4):
                        nc.tensor.matmul(
                            pe[:, :nw],
                            wemb_sb[:, kc, 128 * m:128 * (m + 1)],
                            xT_sb[:, kc, n0:n0 + nw],
                            start=(kc == 0), stop=(kc == 3))
                    nc.scalar.activation(
                        h0T[:, m, n0:n0 + nw], pe[:, :nw],
                        ACTF.Identity, bias=bemb_f[:, m:m + 1], scale=1.0)
                emit_psd(0, i)
                if i == 2:
                    emit_windows()

            # ---- GAT layers
            for l in range(3):
                last = (l == 2)
                hT_in = [h0T, h1T, h2T][l]
                hT_out = [h1T, h2T, None][l]
                ra = RA[l]
                Fh = OUT if last else HID           # h width in stage-A psum
                agg_tiles = SH // 128 if last else NT

                if l > 0:
                    for i in range(3):
                        emit_psd(l, i)
                    emit_windows()

                # stage A; whole-layer attention emitted after tile 2
                for t in range(NT):
                    w = min(128, ra - 128 * t)
                    ps = psp.tile([128, 512], F32, name="ps_A", tag="ps_A")
                    for kc in range(2):
                        nc.tensor.matmul(
                            ps[:w, 0:HID + 2],
                            hT_in[:, kc, 128 * t:128 * t + w],
                            wc_sb[l][:, kc, :],
                            start=(kc == 0), stop=(kc == 1))
                    nc.vector.tensor_copy(H_nm[:w, t, 0:Fh], ps[:w, 0:Fh])
                    nc.vector.tensor_copy(sd2[:w, t, :],
                                          ps[:w, Fh + 1:Fh + 2])
                    if t not in (2, 5, 8) or "attn" in skip:
                        continue
                    # attention chunk (tiles 3c..3c+2): needs w2 + this
                    # chunk's d; no max-subtraction (values bounded)
                    c = t // 3
                    nct = 2 if (last and c == 2) else 3
                    sl = slice(3 * c, 3 * c + nct)
                    w2ap = w2[:]
                    wv = bass.AP(tensor=w2ap.tensor,
                                 offset=w2ap.offset + 384 * c,
                                 ap=[[1057, 128], [128, nct], [1, K17]])
                    nc.vector.tensor_tensor(
                        E[:, sl, :], wv,
                        sd2[:, sl, :].broadcast_to([128, nct, K17]), AT.add)
                    nc.vector.scalar_tensor_tensor(
                        out=E[:, sl, :], in0=E[:, sl, :], scalar=0.2,
                        in1=E[:, sl, :], op0=AT.mult, op1=AT.max)
                    nc.scalar.activation(Cc[:, sl, :], E[:, sl, :], ACTF.Exp)
                    nc.vector.tensor_reduce(
                        out=ssum[:, sl, 0], in_=Cc[:, sl, :],
                        axis=mybir.AxisListType.X, op=AT.add)
                    nc.vector.reciprocal(rs[:, sl, 0], ssum[:, sl, 0])
                    nc.vector.tensor_tensor(
                        Cc[:, sl, :], Cc[:, sl, :],
                        rs[:, sl, :].broadcast_to([128, nct, K17]), AT.mult)
                    # skewed write: cs[128t + p][p + k] = Cc[p, t, k]
                    nc.sync.dma_start(
                        out=bass.AP(tensor=cs, offset=3 * c * 128 * RSK,
                                    ap=[[RSK + 1, 128], [128 * RSK, nct],
                                        [1, K17]]),
                        in_=Cc[:, sl, :])
                    nc.sync.dma_start(
                        out=a_nm[:, sl, :],
                        in_=bass.AP(tensor=cs, offset=3 * c * 128 * RSK,
                                    ap=[[RSK, 128], [128 * RSK, nct],
                                        [1, 128]]))
                    nc.sync.dma_start(
                        out=a_nmh[:, c, 0:32 * nct],
                        in_=bass.AP(tensor=cs,
                                    offset=3 * c * 128 * RSK + 128,
                                    ap=[[RSK, 128], [128 * RSK, nct],
                                        [1, 32]]))

                # transposes + aggregation, chunk by chunk
                for c in range(3) if "agg" not in skip else []:
                    # halo transpose: pbh[32 j + q, p]
                    #   = coef(dst=128(3c+j)+p, src=128(3c+j+1)+q)
                    # (q in [16,32) lands on cs cols 144:160 = zeros)
                    ncc = 2 if (last and c == 2) else 3
                    pbh = psp.tile([128, 128], BF16, name="pbh", tag="ps_T")
                    nc.tensor.transpose(pbh[0:32 * ncc, :],
                                        a_nmh[:, c, 0:32 * ncc], identb[:])
                    for j in range(ncc):
                        nc.vector.tensor_copy(at1a[:, 3 * c + j, :],
                                              pbh[32 * j:32 * j + 16, :])
                    for t in range(3 * c, min(3 * c + 3, agg_tiles)):
                        pa0 = psp.tile([128, 128], BF16, name="pa0",
                                       tag="ps_T")
                        nc.tensor.transpose(pa0[:], a_nm[:, t, :], identb[:])
                        at0 = atp.tile([128, 128], BF16, name="at0",
                                       tag="at0")
                        nc.vector.tensor_copy(at0[:], pa0[:])
                        at1 = at1a[:, t, :]
                        if last:
                            pg = psp.tile([128, OUT], F32, name="ps_G",
                                          tag="ps_G", bufs=3)
                            nc.tensor.matmul(pg[:], at0[:], H_nm[:, t, 0:OUT],
                                             start=True, stop=False)
                            nc.tensor.matmul(pg[:], at1,
                                             H_nm[0:16, t + 1, 0:OUT],
                                             start=False, stop=True)
                            nc.vector.scalar_tensor_tensor(
                                out=out_sb[:, t, :], in0=pg[:], scalar=1.0,
                                in1=b3_bc[:], op0=AT.mult, op1=AT.add)
                            nc.sync.dma_start(
                                out=bass.AP(tensor=out_d,
                                            offset=128 * OUT * t,
                                            ap=[[OUT, 128], [1, OUT]]),
                                in_=out_sb[:, t, :])
                        else:
                            for m in range(2):
                                pf = psp.tile([128, 128], F32, name="ps_G",
                                              tag="ps_G", bufs=3)
                                nc.tensor.matmul(
                                    pf[:], H_nm[:, t, 128 * m:128 * (m + 1)],
                                    at0[:], start=True, stop=False)
                                nc.tensor.matmul(
                                    pf[:],
                                    H_nm[0:16, t + 1, 128 * m:128 * (m + 1)],
                                    at1, start=False, stop=True)
                                nc.scalar.activation(
                                    hT_out[:, m, 128 * t:128 * (t + 1)],
                                    pf[:], ACTF.Identity,
                                    bias=bf[l][:, m:m + 1], scale=1.0)

    nc.compile()
    return nc


def get_nc(repeat=1, skip=()):
    key = ("nc", repeat, tuple(sorted(skip)), os.environ.get("BENCH_SKIP") or "")
    if key not in _CACHE:
        _CACHE[key] = _build(repeat, skip)
    return _CACHE[key]


def prep_in_maps(x, W_emb, b_emb, W_h, asrc_h, adst_h, b_h, W_o, asrc_o,
                 adst_o, b_o):
    x = np.asarray(x, np.float32)
    perm = (OFFSTEP * np.arange(N)) % N         # node id at permuted row a
    x_perm = x[perm]

    def cat(W, a_s, a_d, pad):
        W = np.asarray(W, np.float32)
        cols = [W,
                (W @ np.asarray(a_s, np.float32))[:, None],
                (W @ np.asarray(a_d, np.float32))[:, None]]
        if pad:
            cols.append(np.zeros((W.shape[0], pad), np.float32))
        return np.ascontiguousarray(np.concatenate(cols, 1))

    def sw(a):
        # [K, n] -> [128, K//128, n] with row 128*c + p on (p, c)
        a = np.asarray(a, np.float32)
        return np.ascontiguousarray(
            a.reshape(-1, 128, a.shape[1]).transpose(1, 0, 2))

    shared = {
        "wemb": sw(W_emb),
        "bemb": np.asarray(b_emb, np.float32).reshape(1, HID),
        "wc1": sw(cat(W_h[0], asrc_h[0], adst_h[0], 0)),
        "wc2": sw(cat(W_h[1], asrc_h[1], adst_h[1], 0)),
        "wc3": sw(cat(W_o, asrc_o, adst_o, HID - OUT)),
        "b1": np.asarray(b_h[0], np.float32).reshape(1, HID),
        "b2": np.asarray(b_h[1], np.float32).reshape(1, HID),
        "b3": np.asarray(b_o, np.float32).reshape(1, OUT),
    }
    in_maps = []
    for c in range(NCORES):
        rows = (SH * c + np.arange(RA_EMB)) % N
        xt = sw(x_perm[rows].T)
        in_maps.append({"xT": xt, **shared})
    return in_maps, perm


def assemble(results, perm):
    out_perm = np.concatenate([results[c]["out"] for c in range(NCORES)], 0)
    out = np.empty((N, OUT), np.float32)
    out[perm] = out_perm
    return out


def _pjrt_fn(nc):
    """Memoized variant of bass2jax.run_bass_via_pjrt's multi-core path:
    build the shard_map'd jitted body once per Bass module."""
    key = id(nc)
    if key in _CACHE:
        return _CACHE[key]
    import jax
    import numpy as _np
    from jax.sharding import Mesh, PartitionSpec
    from jax.experimental.shard_map import shard_map
    from concourse import bass2jax, mybir
    bass2jax.install_neuronx_cc_hook()
    n_cores = NCORES
    in_names, out_names, out_avals, zero_outs = [], [], [], []
    pname = nc.partition_id_tensor.name if nc.partition_id_tensor else None
    for alloc in nc.m.functions[0].allocations:
        if not isinstance(alloc, mybir.MemoryLocationSet):
            continue
        name = alloc.memorylocations[0].name
        if alloc.kind == "ExternalInput":
            if name != pname:
                in_names.append(name)
        elif alloc.kind == "ExternalOutput":
            out_names.append(name)
            shape = tuple(alloc.tensor_shape)
            dtype = mybir.dt.np(alloc.dtype)
            out_avals.append(jax.core.ShapedArray(shape, dtype))
            zero_outs.append(_np.zeros(shape, dtype))
    n_params = len(in_names)
    n_outs = len(out_avals)
    all_names = in_names + out_names
    if pname is not None:
        all_names = all_names + [pname]
    donate = tuple(range(n_params, n_params + n_outs))

    def _body(*args):
        operands = list(args)
        if pname is not None:
            operands.append(bass2jax.partition_id_tensor())
        outs = bass2jax._bass_exec_p.bind(
            *operands, out_avals=tuple(out_avals), in_names=tuple(all_names),
            out_names=tuple(out_names), lowering_input_output_aliases=(),
            sim_require_finite=True, sim_require_nnan=True, nc=nc)
        return tuple(outs)

    devices = jax.devices()[:n_cores]
    mesh = Mesh(_np.asarray(devices), ("core",))
    specs = (PartitionSpec("core"),) * (n_params + n_outs)
    out_specs = (PartitionSpec("core"),) * n_outs
    sharded = jax.jit(
        shard_map(_body, mesh=mesh, in_specs=specs, out_specs=out_specs,
                  check_rep=False),
        donate_argnums=donate, keep_unused=True)

    def call(in_maps):
        per_core = [[_np.asarray(m[n]) for n in in_names] for m in in_maps]
        concat_in = [
            _np.concatenate([per_core[c][i] for c in range(n_cores)], axis=0)
            for i in range(n_params)]
        concat_zeros = [
            _np.zeros((n_cores * z.shape[0], *z.shape[1:]), z.dtype)
            for z in zero_outs]
        out_arrs = sharded(*concat_in, *concat_zeros)
        return [
            {name: _np.asarray(out_arrs[i]).reshape(
                n_cores, *out_avals[i].shape)[c]
             for i, name in enumerate(out_names)}
            for c in range(n_cores)]

    call.sharded = sharded
    call.in_names = in_names
    call.out_names = out_names
    call.out_avals = out_avals
    call.zero_outs = zero_outs
    _CACHE[key] = call
    return call


def run(inputs, trace=False, repeat=1, skip=()):
    in_maps, perm = prep_in_maps(
        inputs["x"], inputs["W_emb"], inputs["b_emb"], inputs["W_h"],
        inputs["asrc_h"], inputs["adst_h"], inputs["b_h"], inputs["W_o"],
        inputs["asrc_o"], inputs["adst_o"], inputs["b_o"])
    nc = get_nc(repeat, skip)
    if trace:
        from concourse import bass_utils
        br = bass_utils.run_bass_kernel_spmd(
            nc, in_maps, list(range(NCORES)), trace=True)
        return assemble(br.results, perm), br
    results = _pjrt_fn(nc)(in_maps)

    class _BR:
        exec_time_ns = None
        instructions_and_trace = None
    br = _BR()
    br.results = results
    return assemble(results, perm), br


def kernel(**inputs):
    out, _ = run(inputs)
    return out
